# revision 1
# baseline (speedup 1.0000x reference)
"""Trainium2 Bass kernel for the ArcModel3Phase loss.

Math restructuring (vs the reference):
  Each MC interface term needs logsumexp_n(lpx + lpy + lptx) over N=1024
  samples for each of M points.  Expanding all three log-densities,
      l_nm = A_m + B_n + x_m*(tx_n/sn^2) + y_m*(2 G_n/sn^2)
             + log(1 - exp(-(4/sn^2) y_m G_n))
  The affine part R1_nm = x_m*txp_n + y_m*g1_n + B_n is a matmul, and with
  w_nm = (4/sn^2) y G = R1 - R2 where R2_nm = x_m*txp_n - y_m*g1_n + B_n
  (same matmul, g1 negated),
      sum_n e^{l - b} = sum_n e^{R1-b} - sum_n e^{R2-b}
  for any bound b -- A_m cancels, and b only affects numerics: a drop of
  up to (Ib-Ia)^2/(2 sn^2) ~ 72 below the true max keeps every f32 value
  finite.  w >= 0.2 here, so the subtraction loses < 3 bits.

Three accuracy-preserving device optimizations:
  1. Adaptive sample merging (2nd-order cumulant): a tx-contiguous group
     S of k samples merges as sum_S e^h ~ k e^{mean_h + Var_S(h)/2}.
     h is affine in (x, y), so Var_S(h) is quadratic in (x, y) and is
     carried EXACTLY as 6 extra matmul rows (x^2, y^2, xy, x, y, 1
     coefficients).  Groups grow (up to 64) while every member's
     |h - mean_h| stays <= D_MERGE over the RELEVANT window (|x - tx|
     <= 0.45; beyond it the Gaussian suppression > e^-40 makes the
     group irrelevant for that m).  1024 samples -> ~170 per term;
     residual error is 3rd/4th cumulants, ~2e-4 on the loss.
  2. The mixture only needs SUM_j e^{plane_j}, so one shared bound b per
     m lets all three terms accumulate in a single fused exp+accum pass
     over one concatenated PSUM region (2 ScalarE passes and 2
     accumulator drains per tile instead of 6+6).
  3. Merged samples sorted by G; those with w >= W_SKIP for every m
     (G >= W_SKIP*sn^2/(4 y_min)) contribute < e^-W_SKIP relative to s2
     and are skipped in the R2/e2 pass.

fp32 matmul streams at 1/4 PE rate, so factors are split hi/lo into bf16
(x*t = xh*th + xh*tl + xl*th, ~2^-17 relative; correction rows single
bf16).  The K=13 bf16 matmul streams at full rate.

Per-core layout: M=100000 sharded 8 ways -> 12500, padded to 12544 =
128 partitions x 98 tiles (m = p*98 + t), with a 0/1 mask for the pad.
The mask doubles as the "ones" lhsT row (pad garbage is masked out).
"""
import math

import numpy as np
import ml_dtypes
from scipy.special import erf, erfinv

import concourse.bass as bass
import concourse.tile as tile
from concourse import bacc, mybir
from concourse import bass_isa
from concourse.bass_utils import run_bass_kernel_spmd

WF = 3.0
LOG2PI = math.log(2.0 * math.pi)
M = 100_000
N_MC = 1024
NP = N_MC // 2                 # merged samples per term
N_CORES = 8
M_CORE = M // N_CORES          # 12500
P = 128
T = 98                         # tiles per core; P*T = 12544 >= M_CORE
M_PAD = P * T
W_SKIP = 9.0                   # skip e2 samples with w >= this for all m
K_ROWS = 13
D_MERGE = 16.0                 # max in-window |h - mean_h| within a group
KMAX_GRP = 64
BF16 = ml_dtypes.bfloat16

_graph_cache = {}
_last_results = None


def _split(a):
    hi = a.astype(BF16)
    lo = (a - hi.astype(np.float64)).astype(BF16)
    return hi, lo


def _host_rows(ku, Ia, Ib, sigma_b, sigma_n, logw):
    """Raw per-sample rows for one interface term (float64, tx-sorted)."""
    ku = ku.astype(np.float64)
    sn2 = sigma_n ** 2
    I_min = Ia + 0.5 * (Ib - Ia) * (1.0 + erf(-WF / np.sqrt(2.0)))
    I_diff = (Ib - Ia) * erf(WF / np.sqrt(2.0))
    tx = np.sort(ku * I_diff + I_min)
    ei = erfinv(2.0 * (tx - Ia) / (Ib - Ia) - 1.0)
    G = (Ib - Ia) / np.sqrt(2.0 * np.pi * sigma_b ** 2) * np.exp(-ei ** 2)
    lptx = -np.log(2.0 * WF * (Ib - Ia)) + 0.5 * LOG2PI + ei ** 2
    B = -0.5 * tx ** 2 / sn2 - np.log(G) - G ** 2 / sn2 + lptx
    C0 = (-np.log(sigma_n) - 0.5 * LOG2PI
          + np.log(2.0) - 2.0 * np.log(sigma_n)
          + 0.5 * np.log(2.0 / np.pi) - np.log(2.0)
          - 0.5 * np.log(2.0) + np.log(sigma_n))
    Bp = B + np.log(I_diff) - np.log(N_MC) + logw + C0
    return tx, tx / sn2, 2.0 * G / sn2, Bp, G


def _plan_groups(tx, txp, g1, Bp, xmin, xmax, ymax):
    """Greedy tx-ordered grouping.  A group is acceptable when every
    member's |h_i - mean_h| over the RELEVANT (x, y) window is <= D_MERGE.
    x is restricted to +-0.45 around the group tx mean: beyond that the
    Gaussian e^{-(x-tx)^2/2 sn^2} suppression (> e^-40) makes the group
    irrelevant for that m."""
    groups, i, n = [], 0, len(txp)
    while i < n:
        k = KMAX_GRP
        while k > 1:
            if i + k <= n:
                t, g, b = txp[i:i + k], g1[i:i + k], Bp[i:i + k]
                txm = tx[i:i + k].mean()
                lo = max(xmin, txm - 0.45)
                hi = min(xmax, txm + 0.45)
                dt = t - t.mean()
                db = b - b.mean()
                dg = np.abs(g - g.mean())
                d = (np.maximum(np.abs(dt * lo + db), np.abs(dt * hi + db))
                     + dg * ymax)
                if d.max() <= D_MERGE:
                    break
            k //= 2
        k = max(k, 1)
        groups.append((i, k))
        i += k
    return groups


def _merge_groups(txp, g1, Bp, G, groups):
    """Second-order cumulant merge: sum_S e^h ~ k e^{mean_h + Var_S(h)/2},
    Var_S(h) quadratic in (x, y) -> 6 coefficient rows."""
    out = []
    for i, k in groups:
        t, g, b, gg = txp[i:i + k], g1[i:i + k], Bp[i:i + k], G[i:i + k]
        out.append((t.mean(), g.mean(),
                    b.mean() + math.log(k) + b.var() / 2.0,
                    t.var() / 2.0, g.var() / 2.0,
                    np.mean((t - t.mean()) * (g - g.mean())),
                    np.mean((t - t.mean()) * (b - b.mean())),
                    np.mean((g - g.mean()) * (b - b.mean())),
                    gg.min()))
    return [np.array(v) for v in zip(*out)]


def _pack_rows(tm, gm, Bm, vt, vg, ctg, ctb, cgb, sign, sl):
    """bf16 rhs rows [13, n] for one region.  sign=+1 for R1, -1 for R2.
    lhsT rows: (xh, xh, xl, yh, yh, yl, m, m, x2, y2, xy, xh, yh)."""
    th, tl = _split(tm[sl])
    gh, gl = _split(sign * gm[sl])
    bh, bl = _split(Bm[sl])
    return np.stack([
        th, tl, th,
        gh, gl, gh,
        bh, bl,
        vt[sl].astype(BF16),
        vg[sl].astype(BF16),
        (sign * ctg[sl]).astype(BF16),
        ctb[sl].astype(BF16),
        (sign * cgb[sl]).astype(BF16),
    ]).astype(BF16)


def _bank_slices(offsets):
    """Per-term column ranges, split at PSUM bank (512-col) boundaries."""
    out = []
    for j in range(len(offsets) - 1):
        a, b = offsets[j], offsets[j + 1]
        while a < b:
            c = min(b, (a // 512 + 1) * 512)
            out.append((a, c))
            a = c
    return out


def _build_bass(sigma_n, I1, I2, I3, logw, n1s, nks):
    """Builds the SPMD kernel graph. Scalars are compile-time constants."""
    nc = bacc.Bacc("TRN2", target_bir_lowering=False, debug=False,
                   num_devices=N_CORES)
    dt_ = mybir.dt.float32
    bf = mybir.dt.bfloat16
    f = mybir.ActivationFunctionType
    alu = mybir.AluOpType

    N1T = sum(n1s)                     # combined R1 columns
    nkt = sum(nks)                     # combined R2 columns

    x_d = nc.dram_tensor("x", [M_PAD], dt_, kind="ExternalInput").ap()
    y_d = nc.dram_tensor("y", [M_PAD], dt_, kind="ExternalInput").ap()
    mask_d = nc.dram_tensor("mask", [M_PAD], dt_, kind="ExternalInput").ap()
    lt_d = nc.dram_tensor("lt", [K_ROWS, T, P], bf, kind="ExternalInput").ap()
    rhs1_d = nc.dram_tensor("rhs1", [K_ROWS, N1T], bf,
                            kind="ExternalInput").ap()
    rhs2_d = nc.dram_tensor("rhs2", [K_ROWS, nkt], bf,
                            kind="ExternalInput").ap()
    out_d = nc.dram_tensor("out", [1], dt_, kind="ExternalOutput").ap()

    sn = sigma_n
    ck = (math.log(2.0) - math.lgamma(1.5) - 4.0 * math.log(sn)
          - 0.5 * LOG2PI)

    with tile.TileContext(nc) as tc:
        with (
            tc.tile_pool(name="singles", bufs=1) as singles,
            tc.tile_pool(name="work", bufs=2) as work,
            tc.tile_pool(name="psumA", bufs=2, space="PSUM") as psumA,
            tc.tile_pool(name="psumB", bufs=2, space="PSUM") as psumB,
            tc.tile_pool(name="dump", bufs=3) as dump,
        ):
            # ---- load inputs ----
            xs = singles.tile([P, T], dt_, tag="xs")
            ys = singles.tile([P, T], dt_, tag="ys")
            msk = singles.tile([P, T], dt_, tag="msk")
            nc.sync.dma_start(xs[:], x_d.rearrange("(p t) -> p t", p=P))
            nc.sync.dma_start(ys[:], y_d.rearrange("(p t) -> p t", p=P))
            nc.sync.dma_start(msk[:], mask_d.rearrange("(p t) -> p t", p=P))
            lt = singles.tile([K_ROWS, T, P], bf, tag="lt")
            nc.sync.dma_start(lt[:], lt_d[:])
            rhs1 = singles.tile([K_ROWS, N1T], bf, tag="rhs1")
            nc.sync.dma_start(rhs1[:], rhs1_d[:])
            rhs2 = singles.tile([K_ROWS, nkt], bf, tag="rhs2")
            nc.sync.dma_start(rhs2[:], rhs2_d[:])

            # ---- per-m planes ----
            # all Square activations first, then the single Ln, so the
            # ScalarE table set switches as few times as possible
            sx2h = singles.tile([P, T], dt_, tag="sx2h")
            y2s = singles.tile([P, T], dt_, tag="y2s")
            lny = singles.tile([P, T], dt_, tag="lny")
            nc.scalar.activation(sx2h[:], xs[:], f.Square,
                                 scale=1.0 / (sn * math.sqrt(2.0)))
            nc.scalar.activation(y2s[:], ys[:], f.Square, scale=1.0 / sn)
            qs = []
            for k, I in enumerate((I1, I2, I3)):
                qb = work.tile([P, 1], dt_, tag="qb", name=f"qb{k}")
                nc.vector.memset(qb[:], -I / sn)
                q = singles.tile([P, T], dt_, tag=f"q{k}", name=f"q{k}")
                nc.scalar.activation(q[:], xs[:], f.Square,
                                     scale=1.0 / sn, bias=qb[:])
                qs.append(q)
            nc.scalar.activation(lny[:], ys[:], f.Ln)
            # A = lny - sx2h - y2s
            A = singles.tile([P, T], dt_, tag="A")
            tmpA = work.tile([P, T], dt_, tag="tmpA")
            nc.vector.scalar_tensor_tensor(tmpA[:], sx2h[:], 1.0, y2s[:],
                                           alu.mult, alu.add)
            nc.vector.scalar_tensor_tensor(A[:], tmpA[:], -1.0, lny[:],
                                           alu.mult, alu.add)

            # interior planes -> PL[:, :, 0..2]
            PL = singles.tile([P, T, 4], dt_, tag="PL")
            base = singles.tile([P, T], dt_, tag="base")
            nc.vector.scalar_tensor_tensor(base[:], lny[:], 2.0, y2s[:],
                                           alu.mult, alu.subtract)
            for k in range(3):
                basek = work.tile([P, T], dt_, tag="basek")
                nc.vector.tensor_scalar_add(basek[:], base[:],
                                            ck + float(logw[k]))
                nc.vector.scalar_tensor_tensor(PL[:, :, k], qs[k][:], -0.5,
                                               basek[:], alu.mult, alu.add)

            # ---- hot loop: one fused interface pass per tile ----
            NM = singles.tile([P, T], dt_, tag="NM")
            S1 = singles.tile([P, T], dt_, tag="S1")
            S2 = singles.tile([P, T], dt_, tag="S2")
            sl1 = _bank_slices(np.concatenate([[0], np.cumsum(n1s)]).tolist())
            sl2 = _bank_slices(np.concatenate([[0], np.cumsum(nks)]).tolist())
            for t in range(T):
                lhsT = lt[:, t, :]
                r1 = psumA.tile([P, N1T], dt_, tag="ra")
                for a, b in sl1:
                    nc.tensor.matmul(r1[:, a:b], lhsT, rhs1[:, a:b],
                                     start=True, stop=True)
                r2 = psumB.tile([P, nkt], dt_, tag="rb")
                for a, b in sl2:
                    nc.tensor.matmul(r2[:, a:b], lhsT, rhs2[:, a:b],
                                     start=True, stop=True)
                # coarse (negated) shared upper bound over stride-4 slice
                sub = r1[:].rearrange("p (a b) -> p a b", b=4)[:, :, 0]
                nc.vector.tensor_reduce(NM[:, t: t + 1], sub,
                                        mybir.AxisListType.X, alu.max,
                                        negate=True)
                e1 = dump.tile([P, N1T], dt_, tag="e")
                nc.scalar.activation(e1[:], r1[:], f.Exp,
                                     bias=NM[:, t: t + 1],
                                     accum_out=S1[:, t: t + 1])
                e2 = dump.tile([P, nkt], dt_, tag="e2")
                nc.scalar.activation(e2[:], r2[:], f.Exp,
                                     bias=NM[:, t: t + 1])
                nc.vector.tensor_reduce(S2[:, t: t + 1], e2[:],
                                        mybir.AxisListType.X, alu.add)

            # ---- final mix ----
            # sd = S1 - S2; the interface term joins the mixture as
            # sd * e^{A - NM - mx'} (no ln(sd) pass, and the mix Exp stays
            # in the same ScalarE table set as the hot loop).
            sd = work.tile([P, T], dt_, tag="sd")
            nc.vector.scalar_tensor_tensor(sd[:], S2[:], -1.0, S1[:],
                                           alu.mult, alu.add)
            nc.vector.scalar_tensor_tensor(PL[:, :, 3], NM[:], -1.0,
                                           A[:], alu.mult, alu.add)
            mx6 = singles.tile([P, T, 1], dt_, tag="mx6")
            nc.vector.tensor_reduce(mx6[:], PL[:], mybir.AxisListType.X,
                                    alu.max)
            D = singles.tile([P, T, 4], dt_, tag="D")
            nc.vector.tensor_tensor(D[:], PL[:],
                                    mx6[:].broadcast_to([P, T, 4]),
                                    alu.subtract)
            E = singles.tile([P, T, 4], dt_, tag="E")
            nc.scalar.activation(E[:], D[:], f.Exp)
            # SM = e^{P0-mx'} + e^{P1-mx'} + e^{P2-mx'} + sd * e^{P3-mx'}
            sm3 = singles.tile([P, T, 1], dt_, tag="sm3")
            nc.vector.tensor_reduce(sm3[:], E[:, :, 0:3],
                                    mybir.AxisListType.X, alu.add)
            ifc = work.tile([P, T], dt_, tag="ifc")
            nc.vector.tensor_tensor(ifc[:], E[:, :, 3], sd[:], alu.mult)
            sm = singles.tile([P, T], dt_, tag="sm")
            nc.vector.tensor_tensor(sm[:], sm3[:, :, 0], ifc[:], alu.add)
            lnm = singles.tile([P, T], dt_, tag="lnm")
            nc.scalar.activation(lnm[:], sm[:], f.Ln)
            logmix = singles.tile([P, T], dt_, tag="logmix")
            nc.vector.tensor_tensor(logmix[:], lnm[:], mx6[:, :, 0], alu.add)

            # ---- masked sum over all m; negate on host ----
            colsum = singles.tile([P, 1], dt_, tag="colsum")
            dmp = work.tile([P, T], dt_, tag="dmp")
            nc.vector.scalar_tensor_tensor(dmp[:], logmix[:], 1.0, msk[:],
                                           alu.mult, alu.mult,
                                           accum_out=colsum[:])
            total = singles.tile([P, 1], dt_, tag="total")
            nc.gpsimd.partition_all_reduce(total[:], colsum[:], channels=P,
                                           reduce_op=bass_isa.ReduceOp.add)
            nc.sync.dma_start(out_d.rearrange("(p o) -> p o", p=1),
                              total[0:1, 0:1])

    nc.compile()
    return nc


def _prepare(x, y, ku12, ku23, ku13, sigma_b, sigma_n, I1, I2, I3, w):
    x = np.asarray(x, np.float32)
    y = np.asarray(y, np.float32)
    sigma_b = float(sigma_b)
    sigma_n = float(sigma_n)
    I1, I2, I3 = float(I1), float(I2), float(I3)
    w64 = np.asarray(w, np.float64)
    logw = w64 - (np.log(np.sum(np.exp(w64 - w64.max()))) + w64.max())

    # numeric-safety guard for the coarse shared max bound
    for Ia, Ib in ((I1, I2), (I2, I3), (I1, I3)):
        L = abs(Ib - Ia) * erf(WF / np.sqrt(2.0))
        assert L * L / (2.0 * sigma_n ** 2) < 80.0, "coarse-max bound unsafe"

    y_min = float(y.min())
    g_thresh = W_SKIP * sigma_n ** 2 / (4.0 * max(y_min, 1e-6))
    xmin, xmax = float(x.min()), float(x.max())
    ymax = float(y.max())

    merged = []
    n1s, nks = [], []
    for j, (ku, Ia, Ib) in enumerate(((ku12, I1, I2), (ku23, I2, I3),
                                      (ku13, I1, I3))):
        tx, txp, g1, Bp, G = _host_rows(np.asarray(ku), Ia, Ib, sigma_b,
                                        sigma_n, float(logw[3 + j]))
        groups = _plan_groups(tx, txp, g1, Bp, xmin, xmax, ymax)
        mg = _merge_groups(txp, g1, Bp, G, groups)
        o = np.argsort(mg[8])              # G-sort the merged samples
        mg = [a[o] for a in mg]
        keep = int(np.searchsorted(mg[8], g_thresh))
        nk = min(len(mg[0]), (max(keep, 16) + 15) // 16 * 16)
        # pad the R1 block to a multiple of 4 with dead columns (B=-30000)
        n1 = (len(mg[0]) + 3) // 4 * 4
        pad = n1 - len(mg[0])
        if pad:
            mg = [np.concatenate([a, np.full(pad, -30000.0 if i == 2
                                             else 0.0)])
                  for i, a in enumerate(mg)]
        merged.append(mg)
        n1s.append(n1)
        nks.append(nk)

    rows1 = np.concatenate(
        [_pack_rows(*mg[:8], +1.0, slice(None)) for mg in merged], axis=1)
    rows2 = np.concatenate(
        [_pack_rows(*mg[:8], -1.0, slice(0, nk))
         for mg, nk in zip(merged, nks)], axis=1)

    # lhsT rows (xh,xh,xl, yh,yh,yl, m,m, x2,y2,xy, xh,yh) in [13,T,P]
    mask = np.zeros(M_PAD, np.float32)
    mask[:M_CORE] = 1.0
    mgrid = mask.reshape(P, T).T.astype(BF16)          # [T, P]

    key = (sigma_n, I1, I2, I3, tuple(np.round(logw, 12)),
           tuple(n1s), tuple(nks))
    if key not in _graph_cache:
        _graph_cache[key] = _build_bass(sigma_n, I1, I2, I3, logw, n1s, nks)
    nc = _graph_cache[key]

    in_maps = []
    for i in range(N_CORES):
        xi = np.full(M_PAD, 0.5, np.float64)
        yi = np.full(M_PAD, 0.5, np.float64)
        xi[:M_CORE] = x[i * M_CORE: (i + 1) * M_CORE]
        yi[:M_CORE] = y[i * M_CORE: (i + 1) * M_CORE]
        xh, xl = _split(xi)
        yh, yl = _split(yi)
        x2 = (xi * xi).astype(BF16)
        y2 = (yi * yi).astype(BF16)
        xy = (xi * yi).astype(BF16)
        lt = np.empty((K_ROWS, T, P), BF16)
        planes = (xh, xh, xl, yh, yh, yl, None, None, x2, y2, xy, xh, yh)
        for r, plane in enumerate(planes):
            lt[r] = mgrid if plane is None else plane.reshape(P, T).T
        in_maps.append({"x": xi.astype(np.float32),
                        "y": yi.astype(np.float32), "mask": mask,
                        "lt": lt, "rhs1": rows1, "rhs2": rows2})
    return nc, in_maps


def kernel(x, y, ku12, ku23, ku13, sigma_b, sigma_n, I1, I2, I3, w):
    nc, in_maps = _prepare(x, y, ku12, ku23, ku13, sigma_b, sigma_n,
                           I1, I2, I3, w)
    res = run_bass_kernel_spmd(nc, in_maps, core_ids=list(range(N_CORES)))
    global _last_results
    _last_results = res
    partials = [float(res.results[i]["out"][0]) for i in range(N_CORES)]
    return np.float32(-np.sum(partials))



# revision 2
# speedup vs baseline: 2.5530x; 2.5530x over previous
"""Trainium2 Bass kernel for the ArcModel3Phase loss (y-sorted redesign).

Math: per point m, logmix = ln(sum_j e^{l_j}) over 6 mixture components
(3 interior Gaussians + 3 MC-integrated interface terms of N=1024 samples
each).  Writing l = A(x,y) + h with A = lny - x^2/2sn^2 - y^2/sn^2 and h
affine in (x, y, lny, 1), every component (and the per-m bias) becomes a
column of ONE bf16 matmul over 17 lhsT rows:

  R[p, c] = sum_k lhsT[k, p] rhs[k, c]   -> exp -> segmented row sums.

Device work per point is ~100 columns instead of 3072 thanks to:
  1. Global y-sort (host permutes; the loss is a sum over m, so no
     unpermute).  Each 1024-point block has a narrow y-range, so most MC
     samples are irrelevant to it: a sample contributes only within
     |y - G(tx)| ~ 0.2.  Host prunes per block against a logmix lower
     bound on an x-grid (cutoff e^-PRUNE).
  2. Adaptive sample merging (2nd-order cumulant, exact variance carried
     as 5 extra matmul rows) with a per-block relevance window, plus an
     overshoot guard that keeps each merged column within OCAP of the
     exact logsumexp at window probes (prevents f32 exp overflow and
     bounds the merge error).
  3. The e^{R2} subtraction pass (Bessel 1-e^{-w} expansion) is skipped
     for samples with w = 4yG/sn^2 >= WSKIP for the whole block - almost
     all of them once y is sorted.
  4. The per-m exp bias nu = b - A (b = max of per-component upper
     bounds, a tight cover of max_j l_j) is pure host math, folded into
     the matmul as two hi/lo bf16 rows.  No on-device max pass at all.
  5. Interior components are affine in (x, lny): 3 more columns, two
     lny rows.  The final ln + masked sum runs on host from the DMA'd
     [128, T] mix tile (f64, more accurate than device f32 accum).

One EXP instruction covers a whole batch of tiles (PSUM budget 2048
f32), then two segmented DVE reduces produce S1 (R1+interior) and S2
per tile; mix = S1 - S2.
"""
import math

import numpy as np
import ml_dtypes
from scipy.special import erf, erfinv

import concourse.bass as bass
import concourse.tile as tile
from concourse import bacc, mybir
from concourse.bass_utils import run_bass_kernel_spmd

BF16 = ml_dtypes.bfloat16
WF = 3.0
LOG2PI = math.log(2.0 * math.pi)
M = 100_000
N_MC = 1024
P = 128
N_CORES = 8
BLK = P * N_CORES              # 1024 points per global block
T = (M + BLK - 1) // BLK       # 98 tiles per core
M_PAD = T * BLK
ROWS = 17
DEAD_B = -30000.0

DM = 24.0                      # max in-window |h - mean| within a group
KMAX = 64                      # max group size
PRUNE = 8.0                    # per-block relevance cutoff (e-folds)
WSKIP = 9.0                    # skip R2 columns with w >= this block-wide
OCAP = 2.5                     # max merged-vs-exact LSE overshoot
XWIN = 0.45                    # merge relevance half-window in x
PSUM_BUDGET = 2048             # f32 columns per batch (4 PSUM banks)
BMAX = 16                      # max tiles per batch

_graph_cache = {}
_last_results = None


def _split(a):
    hi = np.asarray(a).astype(BF16)
    lo = (np.asarray(a, np.float64) - hi.astype(np.float64)).astype(BF16)
    return hi, lo


def _host_rows(ku, Ia, Ib, sigma_b, sigma_n, logw):
    """Raw per-sample rows for one interface term (float64, tx-sorted)."""
    ku = np.asarray(ku, np.float64)
    sn2 = sigma_n ** 2
    I_min = Ia + 0.5 * (Ib - Ia) * (1.0 + erf(-WF / np.sqrt(2.0)))
    I_diff = (Ib - Ia) * erf(WF / np.sqrt(2.0))
    tx = np.sort(ku * I_diff + I_min)
    ei = erfinv(2.0 * (tx - Ia) / (Ib - Ia) - 1.0)
    G = (Ib - Ia) / np.sqrt(2.0 * np.pi * sigma_b ** 2) * np.exp(-ei ** 2)
    lptx = -np.log(2.0 * WF * (Ib - Ia)) + 0.5 * LOG2PI + ei ** 2
    B = -0.5 * tx ** 2 / sn2 - np.log(G) - G ** 2 / sn2 + lptx
    C0 = (-np.log(sigma_n) - 0.5 * LOG2PI
          + np.log(2.0) - 2.0 * np.log(sigma_n)
          + 0.5 * np.log(2.0 / np.pi) - np.log(2.0)
          - 0.5 * np.log(2.0) + np.log(sigma_n))
    Bp = B + np.log(I_diff) - np.log(N_MC) + logw + C0
    return tx, tx / sn2, 2.0 * G / sn2, Bp, G


def _raw_l(xg, yv, term, sn2):
    """l_n(xg, yv) for all samples of one term: [X, N]."""
    tx, txp, g1, Bp, G = term
    w = np.minimum(4.0 * yv * G / sn2, 700.0)
    return (Bp[None, :] + xg[:, None] * txp[None, :] + yv * g1[None, :]
            + np.log1p(-np.exp(-w))[None, :]
            + np.log(yv) - 0.5 * (xg[:, None] ** 2) / sn2 - yv * yv / sn2)


def _interior_logp(x, y, I, sn):
    return (math.log(2.0) + 2.0 * np.log(y) - math.lgamma(1.5)
            - 3.0 * math.log(sn) - (y / sn) ** 2
            - math.log(sn) - 0.5 * LOG2PI - 0.5 * ((x - I) / sn) ** 2)


def _plan(x, y, ku12, ku23, ku13, sigma_b, sigma_n, I1, I2, I3, w):
    x = np.asarray(x, np.float64)
    y = np.asarray(y, np.float64)
    sn = float(sigma_n); sb = float(sigma_b)
    I1, I2, I3 = float(I1), float(I2), float(I3)
    w64 = np.asarray(w, np.float64)
    logw = w64 - (np.log(np.sum(np.exp(w64 - w64.max()))) + w64.max())
    sn2 = sn * sn

    terms = [_host_rows(ku, Ia, Ib, sb, sn, float(logw[3 + j]))
             for j, (ku, Ia, Ib) in enumerate(
                 ((ku12, I1, I2), (ku23, I2, I3), (ku13, I1, I3)))]

    # l(x,y) <= lny + c_u: per-sample peak at (tx, G), minus its lny part
    c_u = -1e30
    for tx, txp, g1, Bp, G in terms:
        l_peak = (np.log(G) + 0.5 * tx ** 2 / sn2 + G ** 2 / sn2 + Bp
                  + np.log1p(-np.exp(-np.minimum(4.0 * G * G / sn2, 700.0))))
        c_u = max(c_u, float((l_peak - np.log(G)).max()))

    order = np.argsort(y, kind="stable")
    pad = M_PAD - len(x)
    order_p = np.concatenate([order, np.repeat(order[-1], pad)])
    mask_p = np.concatenate([np.ones(len(x), np.float32),
                             np.zeros(pad, np.float32)])
    ys = y[order_p]

    xmin, xmax = float(x.min()), float(x.max())
    xg = np.linspace(xmin, xmax, 121)

    def logmix_lb(yv):
        mx = np.maximum.reduce([_interior_logp(xg, yv, I, sn) + logw[k]
                                for k, I in enumerate((I1, I2, I3))])
        for term in terms:
            l = _raw_l(xg, yv, term, sn2)
            m2 = l.max(axis=1)
            mx = np.maximum(mx, m2 + np.log(
                np.sum(np.exp(l - m2[:, None]), axis=1)))
        return mx

    tiles = []
    for t in range(T):
        blk = slice(t * BLK, (t + 1) * BLK)
        yb = ys[blk]
        ylo, yhi = float(yb.min()), float(yb.max())
        yprobes = np.linspace(ylo, yhi, 3)
        lmix = np.max([logmix_lb(yv) for yv in yprobes], axis=0)

        tile_terms = []
        for term in terms:
            tx, txp, g1, Bp, G = term
            keep = np.zeros(len(tx), bool)
            for yv in yprobes:
                l = _raw_l(xg, yv, term, sn2)
                keep |= (l - lmix[:, None] >= -PRUNE).any(axis=0)
            yc = np.clip(G, ylo, yhi)
            wv = np.minimum(4.0 * yc * G / sn2, 700.0)
            l = (Bp[None, :] + xg[:, None] * txp[None, :]
                 + (yc * g1)[None, :] + np.log1p(-np.exp(-wv))[None, :]
                 + np.log(yc)[None, :] - 0.5 * (xg[:, None] ** 2) / sn2
                 - (yc * yc)[None, :] / sn2)
            keep |= (l - lmix[:, None] >= -PRUNE).any(axis=0)

            idx = np.where(keep)[0]
            if len(idx) == 0:
                tile_terms.append(None)
                continue
            txk, txpk, g1k, Bpk, Gk = (tx[idx], txp[idx], g1[idx],
                                       Bp[idx], G[idx])
            groups = []
            i = 0
            n = len(idx)
            while i < n:
                k = min(KMAX, n - i)
                while k > 1:
                    tt, gg, bb = txpk[i:i + k], g1k[i:i + k], Bpk[i:i + k]
                    txm = txk[i:i + k].mean()
                    lo = max(xmin, txm - XWIN); hi = min(xmax, txm + XWIN)
                    dt = tt - tt.mean(); db = bb - bb.mean()
                    dg = np.abs(gg - gg.mean())
                    d = (np.maximum(np.abs(dt * lo + db),
                                    np.abs(dt * hi + db)) + dg * yhi)
                    if d.max() <= DM:
                        xp = np.array([lo, 0.5 * (lo + hi), hi])
                        yp = np.array([ylo, yhi])
                        ok = True
                        for sgn in (1.0, -1.0):
                            h = (xp[:, None, None] * tt[None, None, :]
                                 + sgn * yp[None, :, None] * gg[None, None, :]
                                 + bb[None, None, :])
                            mu = h.mean(axis=2)
                            var = h.var(axis=2)
                            mx = h.max(axis=2)
                            lse = mx + np.log(
                                np.exp(h - mx[:, :, None]).sum(axis=2))
                            if ((mu + var / 2.0 + math.log(k) - lse).max()
                                    > OCAP):
                                ok = False
                                break
                        if ok:
                            break
                    k = k - 1 if k <= 4 else int(k * 0.7)
                k = max(k, 1)
                groups.append((i, k))
                i += k
            mg = []
            for i0, k in groups:
                tt, gg, bb = txpk[i0:i0 + k], g1k[i0:i0 + k], Bpk[i0:i0 + k]
                mg.append((tt.mean(), gg.mean(),
                           bb.mean() + math.log(k) + bb.var() / 2.0,
                           tt.var() / 2.0, gg.var() / 2.0,
                           np.mean((tt - tt.mean()) * (gg - gg.mean())),
                           np.mean((tt - tt.mean()) * (bb - bb.mean())),
                           np.mean((gg - gg.mean()) * (bb - bb.mean())),
                           Gk[i0:i0 + k].min()))
            cols = [np.array(v) for v in zip(*mg)]
            keep2 = cols[8] * 4.0 * max(ylo, 1e-9) / sn2 < WSKIP
            tile_terms.append((cols, keep2))

        c1 = sum(len(tt[0][0]) for tt in tile_terms if tt)
        c2 = sum(int(tt[1].sum()) for tt in tile_terms if tt)
        tiles.append({"ylo": ylo, "yhi": yhi, "terms": tile_terms,
                      "c1": c1, "c2": c2})

    batches = []
    bstart = 0
    while bstart < T:
        bend = bstart + 1
        best = bstart + 1
        while bend <= T:
            c1m = max(tl["c1"] for tl in tiles[bstart:bend])
            c2m = max(tl["c2"] for tl in tiles[bstart:bend])
            Bn = bend - bstart
            if Bn * (c1m + 3 + c2m) > PSUM_BUDGET or Bn > BMAX:
                break
            best = bend
            bend += 1
        bend = best
        c1m = max(tl["c1"] for tl in tiles[bstart:bend])
        c2m = max(tl["c2"] for tl in tiles[bstart:bend])
        batches.append((bstart, bend, c1m, c2m))
        bstart = bend

    return {"order_p": order_p, "mask_p": mask_p, "tiles": tiles,
            "batches": batches, "logw": logw, "sn": sn, "c_u": c_u,
            "I": (I1, I2, I3)}


def _term_block(cols, sel, sgn):
    """rhs columns [ROWS, n] for one term's merged samples."""
    tm, gm, Bm, vt, vg, ctg, ctb, cgb, _g = cols
    th, tl = _split(tm[sel]); gh, gl = _split(sgn * gm[sel])
    bh, bl = _split(Bm[sel])
    n = len(th)
    one = np.ones(n, BF16)
    z = np.zeros(n, BF16)
    return np.stack([
        th, tl, th, gh, gl, gh, bh, bl,
        vt[sel].astype(BF16), vg[sel].astype(BF16),
        (sgn * ctg[sel]).astype(BF16), ctb[sel].astype(BF16),
        (sgn * cgb[sel]).astype(BF16),
        one, one, z, z,
    ]).astype(BF16)


def _pack(pl, x, y):
    """Build rhs [ROWS, NT] (shared), per-core lt [ROWS,T,P], mask, bvec."""
    sn = pl["sn"]; sn2 = sn * sn
    I1, I2, I3 = pl["I"]
    logw = pl["logw"]
    xs = np.asarray(x, np.float64)[pl["order_p"]]
    ysrt = np.asarray(y, np.float64)[pl["order_p"]]

    lny = np.log(ysrt)
    A = lny - 0.5 * (xs / sn) ** 2 - (ysrt / sn) ** 2
    Ps = [_interior_logp(xs, ysrt, I, sn) + logw[k]
          for k, I in enumerate((I1, I2, I3))]
    b_m = np.maximum.reduce(Ps + [lny + pl["c_u"]])
    nu64 = b_m - A
    nmh, nml = _split(-nu64)
    nu_use = -(nmh.astype(np.float64) + nml.astype(np.float64))
    bvec = (A + nu_use)
    lh, ll = _split(lny)

    ck = (math.log(2.0) - math.lgamma(1.5) - 4.0 * math.log(sn)
          - 0.5 * LOG2PI)
    intcols = np.zeros((ROWS, 3), BF16)
    for k, I in enumerate((I1, I2, I3)):
        tih, til = _split(np.array([I / sn2]))
        csth, cstl = _split(np.array([ck + logw[k] - 0.5 * I * I / sn2]))
        col = np.zeros(ROWS, BF16)
        col[0], col[1], col[2] = tih[0], til[0], tih[0]
        col[6], col[7] = csth[0], cstl[0]
        col[13] = col[14] = col[15] = col[16] = BF16(1.0)
        intcols[:, k] = col
    deadcol = np.zeros(ROWS, BF16)
    deadcol[6] = BF16(DEAD_B)

    # rhs: batch-major, per tile [C1' R1-cols | 3 interior | C2' R2-cols]
    rhs_parts = []
    meta = []
    off = 0
    for (t0, t1, C1, C2) in pl["batches"]:
        CW = C1 + 3 + C2
        for t in range(t0, t1):
            tt = pl["tiles"][t]
            blocks = []
            n1 = 0
            for j in range(3):
                ter = tt["terms"][j]
                if ter is None:
                    continue
                cols, keep2 = ter
                blk = _term_block(cols, slice(None), 1.0)
                blocks.append(blk)
                n1 += blk.shape[1]
            if n1 < C1:
                blocks.append(np.repeat(deadcol[:, None], C1 - n1, axis=1))
            blocks.append(intcols)
            n2 = 0
            for j in range(3):
                ter = tt["terms"][j]
                if ter is None:
                    continue
                cols, keep2 = ter
                if keep2.any():
                    blk = _term_block(cols, np.where(keep2)[0], -1.0)
                    blocks.append(blk)
                    n2 += blk.shape[1]
            if n2 < C2:
                blocks.append(np.repeat(deadcol[:, None], C2 - n2, axis=1))
            rhs_parts.append(np.concatenate(blocks, axis=1))
        meta.append((t0, t1, C1, C2, off))
        off += (t1 - t0) * CW
    rhs = np.concatenate(rhs_parts, axis=1).astype(BF16)
    NT = rhs.shape[1]
    assert NT == off

    xh, xl = _split(xs); yh, yl = _split(ysrt)
    planes = [
        xh, xh, xl, yh, yh, yl,
        np.ones(M_PAD, BF16), np.ones(M_PAD, BF16),
        (xs * xs).astype(BF16), (ysrt * ysrt).astype(BF16),
        (xs * ysrt).astype(BF16), xh, yh, nmh, nml, lh, ll,
    ]

    # safety: packed R - nu must stay well under f32 exp overflow
    pf = np.stack([p.astype(np.float32) for p in planes]).astype(np.float64)
    rf = rhs.astype(np.float64)
    vmax = -1e30
    for (t0, t1, C1, C2, off0) in meta:
        CW = C1 + 3 + C2
        for s, t in enumerate(range(t0, t1)):
            blk = slice(t * BLK, (t + 1) * BLK)
            Rt = pf[:, blk].T @ rf[:, off0 + s * CW: off0 + (s + 1) * CW]
            vmax = max(vmax, float(Rt.max()))
    assert vmax < 60.0, f"exp overflow risk: max(R-nu)={vmax:.1f}"

    in_maps = []
    cores_bvec = []
    cores_mask = []
    for i in range(N_CORES):
        lt = np.empty((ROWS, T, P), BF16)
        for r in range(ROWS):
            lt[r] = planes[r].reshape(T, N_CORES, P)[:, i, :]
        mask_i = np.ascontiguousarray(
            pl["mask_p"].reshape(T, N_CORES, P)[:, i, :].T).astype(np.float32)
        bvec_i = bvec.reshape(T, N_CORES, P)[:, i, :].T
        in_maps.append({"lt": lt, "rhs": rhs})
        cores_mask.append(mask_i)
        cores_bvec.append(bvec_i)
    return in_maps, meta, NT, cores_mask, cores_bvec


def _build(meta, NT):
    nc = bacc.Bacc("TRN2", target_bir_lowering=False, debug=False,
                   num_devices=N_CORES)
    dt_ = mybir.dt.float32
    bf = mybir.dt.bfloat16
    f = mybir.ActivationFunctionType
    alu = mybir.AluOpType

    lt_d = nc.dram_tensor("lt", [ROWS, T, P], bf, kind="ExternalInput").ap()
    rhs_d = nc.dram_tensor("rhs", [ROWS, NT], bf, kind="ExternalInput").ap()
    out_d = nc.dram_tensor("out", [P, T], dt_, kind="ExternalOutput").ap()

    def bank_slices(a, b):
        out = []
        while a < b:
            c = min(b, (a // 512 + 1) * 512)
            out.append((a, c))
            a = c
        return out

    with tile.TileContext(nc) as tc:
        with (
            tc.tile_pool(name="singles", bufs=1) as singles,
            tc.tile_pool(name="work", bufs=2) as work,
            tc.tile_pool(name="psum", bufs=2, space="PSUM") as psum_pool,
            tc.tile_pool(name="dump", bufs=2) as dump_pool,
        ):
            lt = singles.tile([ROWS, T, P], bf, tag="lt")
            # chunked loads so early batches start before the full DMA
            nb = len(meta)
            edges = [meta[0][0]] + [m[1] for m in meta]
            for ci in range(nb):
                a, b = edges[ci], edges[ci + 1]
                nc.sync.dma_start(lt[:, a:b, :], lt_d[:, a:b, :])
            rhs = singles.tile([ROWS, NT], bf, tag="rhs")
            for (t0, t1, C1, C2, off0) in meta:
                w = (t1 - t0) * (C1 + 3 + C2)
                nc.sync.dma_start(rhs[:, off0:off0 + w],
                                  rhs_d[:, off0:off0 + w])

            S1 = singles.tile([P, T], dt_, tag="S1")
            S2 = singles.tile([P, T], dt_, tag="S2")
            nc.vector.memset(S2[:], 0.0)

            for (t0, t1, C1, C2, off0) in meta:
                CW = C1 + 3 + C2
                Bn = t1 - t0
                Wb = Bn * CW
                ps = psum_pool.tile([P, Wb], dt_, tag="ps", name="ps")
                dp = dump_pool.tile([P, Wb], dt_, tag="dp", name="dp")
                for s in range(Bn):
                    for a, b in bank_slices(s * CW, (s + 1) * CW):
                        nc.tensor.matmul(ps[:, a:b], lt[:, t0 + s, :],
                                         rhs[:, off0 + a:off0 + b],
                                         start=True, stop=True)
                nc.scalar.activation(dp[:], ps[:], f.Exp)
                r3 = dp.rearrange("p (b c) -> p b c", c=CW)
                nc.vector.tensor_reduce(S1[:, t0:t1], r3[:, :, 0:C1 + 3],
                                        mybir.AxisListType.X, alu.add)
                if C2 > 0:
                    nc.vector.tensor_reduce(S2[:, t0:t1],
                                            r3[:, :, C1 + 3:CW],
                                            mybir.AxisListType.X, alu.add)

            sd = work.tile([P, T], dt_, tag="sd")
            nc.vector.scalar_tensor_tensor(sd[:], S2[:], -1.0, S1[:],
                                           alu.mult, alu.add)
            nc.sync.dma_start(out_d, sd[:])

    nc.compile()
    return nc


def kernel(x, y, ku12, ku23, ku13, sigma_b, sigma_n, I1, I2, I3, w):
    pl = _plan(x, y, ku12, ku23, ku13, sigma_b, sigma_n, I1, I2, I3, w)
    in_maps, meta, NT, cores_mask, cores_bvec = _pack(pl, x, y)

    key = (NT, tuple((m[0], m[1], m[2], m[3]) for m in meta))
    if key not in _graph_cache:
        _graph_cache[key] = _build(meta, NT)
    nc = _graph_cache[key]

    res = run_bass_kernel_spmd(nc, in_maps, core_ids=list(range(N_CORES)))
    global _last_results
    _last_results = res

    loss = 0.0
    for i in range(N_CORES):
        sd = np.asarray(res.results[i]["out"], np.float64)
        lm = np.log(np.maximum(sd, 1e-300)) + cores_bvec[i]
        loss += float((lm * cores_mask[i]).sum())
    return np.float32(-loss)


# revision 4
# speedup vs baseline: 3.8739x; 1.5174x over previous
"""Trainium2 Bass kernel for the ArcModel3Phase loss (y-sorted redesign).

Math: per point m, logmix = ln(sum_j e^{l_j}) over 6 mixture components
(3 interior Gaussians + 3 MC-integrated interface terms of N=1024 samples
each).  Writing l = A(x,y) + h with A = lny - x^2/2sn^2 - y^2/sn^2 and h
affine in (x, y, lny, 1), every component (and the per-m bias) becomes a
column of ONE bf16 matmul over 17 lhsT rows:

  R[p, c] = sum_k lhsT[k, p] rhs[k, c]   -> exp -> segmented row sums.

Device work per point is ~100 columns instead of 3072 thanks to:
  1. Global y-sort (host permutes; the loss is a sum over m, so no
     unpermute).  Each 1024-point block has a narrow y-range, so most MC
     samples are irrelevant to it: a sample contributes only within
     |y - G(tx)| ~ 0.2.  Host prunes per block against a logmix lower
     bound on an x-grid (cutoff e^-PRUNE).
  2. Adaptive sample merging (2nd-order cumulant, exact variance carried
     as 5 extra matmul rows) with a per-block relevance window, plus an
     overshoot guard that keeps each merged column within OCAP of the
     exact logsumexp at window probes (prevents f32 exp overflow and
     bounds the merge error).
  3. The e^{R2} subtraction pass (Bessel 1-e^{-w} expansion) is skipped
     for samples with w = 4yG/sn^2 >= WSKIP for the whole block - almost
     all of them once y is sorted.
  4. The per-m exp bias nu = b - A (b = max of per-component upper
     bounds, a tight cover of max_j l_j) is pure host math, folded into
     the matmul as two hi/lo bf16 rows.  No on-device max pass at all.
  5. Interior components are affine in (x, lny): 3 more columns, two
     lny rows.  The final ln + masked sum runs on host from the DMA'd
     [128, T] mix tile (f64, more accurate than device f32 accum).

One EXP instruction covers a whole batch of tiles (PSUM budget 2048
f32), then two segmented DVE reduces produce S1 (R1+interior) and S2
per tile; mix = S1 - S2.
"""
import math

import numpy as np
import ml_dtypes
from scipy.special import erf, erfinv

import concourse.bass as bass
import concourse.tile as tile
from concourse import bacc, mybir
from concourse import bass_isa
from concourse.bass_utils import run_bass_kernel_spmd

BF16 = ml_dtypes.bfloat16
WF = 3.0
LOG2PI = math.log(2.0 * math.pi)
M = 100_000
N_MC = 1024
P = 128
N_CORES = 8
BLK = P * N_CORES              # 1024 points per global block
T = (M + BLK - 1) // BLK       # 98 tiles per core
M_PAD = T * BLK
ROWS = 15
DEAD_B = -30000.0

DM = 24.0                      # max in-window |h - mean| within a group
KMAX = 64                      # max group size
PRUNE = 8.0                    # per-block relevance cutoff (e-folds)
WSKIP = 9.0                    # skip R2 columns with w >= this block-wide
OCAP = 2.5                     # max merged-vs-exact LSE overshoot
XWIN = 0.45                    # merge relevance half-window in x
PSUM_BUDGET = 2048             # f32 columns per batch (4 PSUM banks)
BMAX = 16                      # max tiles per batch

_graph_cache = {}
_last_results = None


def _split(a):
    hi = np.asarray(a).astype(BF16)
    lo = (np.asarray(a, np.float64) - hi.astype(np.float64)).astype(BF16)
    return hi, lo


def _host_rows(ku, Ia, Ib, sigma_b, sigma_n, logw):
    """Raw per-sample rows for one interface term (float64, tx-sorted)."""
    ku = np.asarray(ku, np.float64)
    sn2 = sigma_n ** 2
    I_min = Ia + 0.5 * (Ib - Ia) * (1.0 + erf(-WF / np.sqrt(2.0)))
    I_diff = (Ib - Ia) * erf(WF / np.sqrt(2.0))
    tx = np.sort(ku * I_diff + I_min)
    ei = erfinv(2.0 * (tx - Ia) / (Ib - Ia) - 1.0)
    G = (Ib - Ia) / np.sqrt(2.0 * np.pi * sigma_b ** 2) * np.exp(-ei ** 2)
    lptx = -np.log(2.0 * WF * (Ib - Ia)) + 0.5 * LOG2PI + ei ** 2
    B = -0.5 * tx ** 2 / sn2 - np.log(G) - G ** 2 / sn2 + lptx
    C0 = (-np.log(sigma_n) - 0.5 * LOG2PI
          + np.log(2.0) - 2.0 * np.log(sigma_n)
          + 0.5 * np.log(2.0 / np.pi) - np.log(2.0)
          - 0.5 * np.log(2.0) + np.log(sigma_n))
    Bp = B + np.log(I_diff) - np.log(N_MC) + logw + C0
    return tx, tx / sn2, 2.0 * G / sn2, Bp, G


def _raw_l(xg, yv, term, sn2):
    """l_n(xg, yv) for all samples of one term: [X, N]."""
    tx, txp, g1, Bp, G = term
    w = np.minimum(4.0 * yv * G / sn2, 700.0)
    return (Bp[None, :] + xg[:, None] * txp[None, :] + yv * g1[None, :]
            + np.log1p(-np.exp(-w))[None, :]
            + np.log(yv) - 0.5 * (xg[:, None] ** 2) / sn2 - yv * yv / sn2)


def _interior_logp(x, y, I, sn):
    return (math.log(2.0) + 2.0 * np.log(y) - math.lgamma(1.5)
            - 3.0 * math.log(sn) - (y / sn) ** 2
            - math.log(sn) - 0.5 * LOG2PI - 0.5 * ((x - I) / sn) ** 2)


def _plan(x, y, ku12, ku23, ku13, sigma_b, sigma_n, I1, I2, I3, w):
    x = np.asarray(x, np.float64)
    y = np.asarray(y, np.float64)
    sn = float(sigma_n); sb = float(sigma_b)
    I1, I2, I3 = float(I1), float(I2), float(I3)
    w64 = np.asarray(w, np.float64)
    logw = w64 - (np.log(np.sum(np.exp(w64 - w64.max()))) + w64.max())
    sn2 = sn * sn

    terms = [_host_rows(ku, Ia, Ib, sb, sn, float(logw[3 + j]))
             for j, (ku, Ia, Ib) in enumerate(
                 ((ku12, I1, I2), (ku23, I2, I3), (ku13, I1, I3)))]

    # l(x,y) <= lny + c_u: per-sample peak at (tx, G), minus its lny part
    c_u = -1e30
    for tx, txp, g1, Bp, G in terms:
        l_peak = (np.log(G) + 0.5 * tx ** 2 / sn2 + G ** 2 / sn2 + Bp
                  + np.log1p(-np.exp(-np.minimum(4.0 * G * G / sn2, 700.0))))
        c_u = max(c_u, float((l_peak - np.log(G)).max()))

    order = np.argsort(y, kind="stable")
    pad = M_PAD - len(x)
    order_p = np.concatenate([order, np.repeat(order[-1], pad)])
    mask_p = np.concatenate([np.ones(len(x), np.float32),
                             np.zeros(pad, np.float32)])
    ys = y[order_p]

    xmin, xmax = float(x.min()), float(x.max())
    xg = np.linspace(xmin, xmax, 121)

    def logmix_lb(yv):
        mx = np.maximum.reduce([_interior_logp(xg, yv, I, sn) + logw[k]
                                for k, I in enumerate((I1, I2, I3))])
        for term in terms:
            l = _raw_l(xg, yv, term, sn2)
            m2 = l.max(axis=1)
            mx = np.maximum(mx, m2 + np.log(
                np.sum(np.exp(l - m2[:, None]), axis=1)))
        return mx

    tiles = []
    for t in range(T):
        blk = slice(t * BLK, (t + 1) * BLK)
        yb = ys[blk]
        ylo, yhi = float(yb.min()), float(yb.max())
        yprobes = np.linspace(ylo, yhi, 3)
        lmix = np.max([logmix_lb(yv) for yv in yprobes], axis=0)

        tile_terms = []
        for term in terms:
            tx, txp, g1, Bp, G = term
            keep = np.zeros(len(tx), bool)
            for yv in yprobes:
                l = _raw_l(xg, yv, term, sn2)
                keep |= (l - lmix[:, None] >= -PRUNE).any(axis=0)
            yc = np.clip(G, ylo, yhi)
            wv = np.minimum(4.0 * yc * G / sn2, 700.0)
            l = (Bp[None, :] + xg[:, None] * txp[None, :]
                 + (yc * g1)[None, :] + np.log1p(-np.exp(-wv))[None, :]
                 + np.log(yc)[None, :] - 0.5 * (xg[:, None] ** 2) / sn2
                 - (yc * yc)[None, :] / sn2)
            keep |= (l - lmix[:, None] >= -PRUNE).any(axis=0)

            idx = np.where(keep)[0]
            if len(idx) == 0:
                tile_terms.append(None)
                continue
            txk, txpk, g1k, Bpk, Gk = (tx[idx], txp[idx], g1[idx],
                                       Bp[idx], G[idx])
            groups = []
            i = 0
            n = len(idx)
            while i < n:
                k = min(KMAX, n - i)
                while k > 1:
                    tt, gg, bb = txpk[i:i + k], g1k[i:i + k], Bpk[i:i + k]
                    txm = txk[i:i + k].mean()
                    lo = max(xmin, txm - XWIN); hi = min(xmax, txm + XWIN)
                    dt = tt - tt.mean(); db = bb - bb.mean()
                    dg = np.abs(gg - gg.mean())
                    d = (np.maximum(np.abs(dt * lo + db),
                                    np.abs(dt * hi + db)) + dg * yhi)
                    if d.max() <= DM:
                        xp = np.array([lo, 0.5 * (lo + hi), hi])
                        yp = np.array([ylo, yhi])
                        ok = True
                        for sgn in (1.0, -1.0):
                            h = (xp[:, None, None] * tt[None, None, :]
                                 + sgn * yp[None, :, None] * gg[None, None, :]
                                 + bb[None, None, :])
                            mu = h.mean(axis=2)
                            var = h.var(axis=2)
                            mx = h.max(axis=2)
                            lse = mx + np.log(
                                np.exp(h - mx[:, :, None]).sum(axis=2))
                            if ((mu + var / 2.0 + math.log(k) - lse).max()
                                    > OCAP):
                                ok = False
                                break
                        if ok:
                            break
                    k = k - 1 if k <= 4 else int(k * 0.7)
                k = max(k, 1)
                groups.append((i, k))
                i += k
            mg = []
            for i0, k in groups:
                tt, gg, bb = txpk[i0:i0 + k], g1k[i0:i0 + k], Bpk[i0:i0 + k]
                mg.append((tt.mean(), gg.mean(),
                           bb.mean() + math.log(k) + bb.var() / 2.0,
                           tt.var() / 2.0, gg.var() / 2.0,
                           np.mean((tt - tt.mean()) * (gg - gg.mean())),
                           np.mean((tt - tt.mean()) * (bb - bb.mean())),
                           np.mean((gg - gg.mean()) * (bb - bb.mean())),
                           Gk[i0:i0 + k].min()))
            cols = [np.array(v) for v in zip(*mg)]
            keep2 = cols[8] * 4.0 * max(ylo, 1e-9) / sn2 < WSKIP
            tile_terms.append((cols, keep2))

        c1 = sum(len(tt[0][0]) for tt in tile_terms if tt)
        c2 = sum(int(tt[1].sum()) for tt in tile_terms if tt)
        tiles.append({"ylo": ylo, "yhi": yhi, "terms": tile_terms,
                      "c1": c1, "c2": c2})

    batches = []
    bstart = 0
    while bstart < T:
        bend = bstart + 1
        best = bstart + 1
        while bend <= T:
            c1m = max(tl["c1"] for tl in tiles[bstart:bend])
            c2m = max(tl["c2"] for tl in tiles[bstart:bend])
            Bn = bend - bstart
            if Bn * (c1m + 3 + c2m) > PSUM_BUDGET or Bn > BMAX:
                break
            best = bend
            bend += 1
        bend = best
        c1m = max(tl["c1"] for tl in tiles[bstart:bend])
        c2m = max(tl["c2"] for tl in tiles[bstart:bend])
        batches.append((bstart, bend, c1m, c2m))
        bstart = bend

    return {"order_p": order_p, "mask_p": mask_p, "tiles": tiles,
            "batches": batches, "logw": logw, "sn": sn, "c_u": c_u,
            "I": (I1, I2, I3)}


def _term_block(cols, sel, sgn):
    """rhs columns [ROWS, n] for one term's merged samples.  The ctb/cgb
    covariance corrections are folded into the t/g rows before the hi/lo
    split (they pair with the same x/y lhsT planes)."""
    tm, gm, Bm, vt, vg, ctg, ctb, cgb, _g = cols
    th, tl = _split(tm[sel] + ctb[sel])
    gh, gl = _split(sgn * (gm[sel] + cgb[sel]))
    bh, bl = _split(Bm[sel])
    n = len(th)
    one = np.ones(n, BF16)
    z = np.zeros(n, BF16)
    return np.stack([
        th, tl, th, gh, gl, gh, bh, bl,
        vt[sel].astype(BF16), vg[sel].astype(BF16),
        (sgn * ctg[sel]).astype(BF16),
        one, one, z, z,
    ]).astype(BF16)


def _pack(pl, x, y):
    """Build rhs [ROWS, NT] (shared), per-core lt [ROWS,T,P], mask, bvec."""
    sn = pl["sn"]; sn2 = sn * sn
    I1, I2, I3 = pl["I"]
    logw = pl["logw"]
    xs = np.asarray(x, np.float64)[pl["order_p"]]
    ysrt = np.asarray(y, np.float64)[pl["order_p"]]

    lny = np.log(ysrt)
    A = lny - 0.5 * (xs / sn) ** 2 - (ysrt / sn) ** 2
    Ps = [_interior_logp(xs, ysrt, I, sn) + logw[k]
          for k, I in enumerate((I1, I2, I3))]
    b_m = np.maximum.reduce(Ps + [lny + pl["c_u"]])
    nu64 = b_m - A
    nmh, nml = _split(-nu64)
    nu_use = -(nmh.astype(np.float64) + nml.astype(np.float64))
    bvec = (A + nu_use)
    lh, ll = _split(lny)

    ck = (math.log(2.0) - math.lgamma(1.5) - 4.0 * math.log(sn)
          - 0.5 * LOG2PI)
    intcols = np.zeros((ROWS, 3), BF16)
    for k, I in enumerate((I1, I2, I3)):
        tih, til = _split(np.array([I / sn2]))
        csth, cstl = _split(np.array([ck + logw[k] - 0.5 * I * I / sn2]))
        col = np.zeros(ROWS, BF16)
        col[0], col[1], col[2] = tih[0], til[0], tih[0]
        col[6], col[7] = csth[0], cstl[0]
        col[11] = col[12] = col[13] = col[14] = BF16(1.0)
        intcols[:, k] = col
    deadcol = np.zeros(ROWS, BF16)
    deadcol[6] = BF16(DEAD_B)

    # rhs: batch-major, per tile [C1' R1-cols | 3 interior | C2' R2-cols]
    rhs_parts = []
    meta = []
    off = 0
    for (t0, t1, C1, C2) in pl["batches"]:
        CW = C1 + 3 + C2
        for t in range(t0, t1):
            tt = pl["tiles"][t]
            blocks = []
            n1 = 0
            for j in range(3):
                ter = tt["terms"][j]
                if ter is None:
                    continue
                cols, keep2 = ter
                blk = _term_block(cols, slice(None), 1.0)
                blocks.append(blk)
                n1 += blk.shape[1]
            if n1 < C1:
                blocks.append(np.repeat(deadcol[:, None], C1 - n1, axis=1))
            blocks.append(intcols)
            n2 = 0
            for j in range(3):
                ter = tt["terms"][j]
                if ter is None:
                    continue
                cols, keep2 = ter
                if keep2.any():
                    blk = _term_block(cols, np.where(keep2)[0], -1.0)
                    blocks.append(blk)
                    n2 += blk.shape[1]
            if n2 < C2:
                blocks.append(np.repeat(deadcol[:, None], C2 - n2, axis=1))
            rhs_parts.append(np.concatenate(blocks, axis=1))
        meta.append((t0, t1, C1, C2, off))
        off += (t1 - t0) * CW
    rhs = np.concatenate(rhs_parts, axis=1).astype(BF16)
    NT = rhs.shape[1]
    assert NT == off

    xh, xl = _split(xs); yh, yl = _split(ysrt)
    planes = [
        xh, xh, xl, yh, yh, yl,
        np.ones(M_PAD, BF16), np.ones(M_PAD, BF16),
        (xs * xs).astype(BF16), (ysrt * ysrt).astype(BF16),
        (xs * ysrt).astype(BF16), nmh, nml, lh, ll,
    ]
    # pad slots: all-zero planes -> R = 0 for every column -> the slot
    # contributes exactly ln(C1'+3-C2') to the device sum (host-corrected)
    padm = pl["mask_p"] == 0.0
    planes = [np.where(padm, np.zeros(1, BF16), p).astype(BF16)
              for p in planes]

    # safety: packed R - nu must stay well under f32 exp overflow
    pf = np.stack([p.astype(np.float32) for p in planes]).astype(np.float64)
    rf = rhs.astype(np.float64)
    vmax = -1e30
    for (t0, t1, C1, C2, off0) in meta:
        CW = C1 + 3 + C2
        for s, t in enumerate(range(t0, t1)):
            blk = slice(t * BLK, (t + 1) * BLK)
            Rt = pf[:, blk].T @ rf[:, off0 + s * CW: off0 + (s + 1) * CW]
            vmax = max(vmax, float(Rt.max()))
    assert vmax < 60.0, f"exp overflow risk: max(R-nu)={vmax:.1f}"

    in_maps = []
    cores_bvec = []
    cores_mask = []
    for i in range(N_CORES):
        lt = np.empty((ROWS, T, P), BF16)
        for r in range(ROWS):
            lt[r] = planes[r].reshape(T, N_CORES, P)[:, i, :]
        mask_i = np.ascontiguousarray(
            pl["mask_p"].reshape(T, N_CORES, P)[:, i, :].T).astype(np.float32)
        bvec_i = bvec.reshape(T, N_CORES, P)[:, i, :].T
        in_maps.append({"lt": lt, "rhs": rhs})
        cores_mask.append(mask_i)
        cores_bvec.append(bvec_i)
    return in_maps, meta, NT, cores_mask, cores_bvec


def _build(meta, NT):
    nc = bacc.Bacc("TRN2", target_bir_lowering=False, debug=False,
                   num_devices=N_CORES)
    dt_ = mybir.dt.float32
    bf = mybir.dt.bfloat16
    f = mybir.ActivationFunctionType
    alu = mybir.AluOpType

    lt_d = nc.dram_tensor("lt", [ROWS, T, P], bf, kind="ExternalInput").ap()
    rhs_d = nc.dram_tensor("rhs", [ROWS, NT], bf, kind="ExternalInput").ap()
    out_d = nc.dram_tensor("out", [1], dt_, kind="ExternalOutput").ap()

    def bank_slices(a, b):
        out = []
        while a < b:
            c = min(b, (a // 512 + 1) * 512)
            out.append((a, c))
            a = c
        return out

    with tile.TileContext(nc) as tc:
        with (
            tc.tile_pool(name="singles", bufs=1) as singles,
            tc.tile_pool(name="work", bufs=2) as work,
            tc.tile_pool(name="psum", bufs=2, space="PSUM") as psum_pool,
            tc.tile_pool(name="dump", bufs=2) as dump_pool,
        ):
            lt = singles.tile([ROWS, T, P], bf, tag="lt")
            rhs = singles.tile([ROWS, NT], bf, tag="rhs")
            # chunked loads, round-robin across the three DMA-capable
            # engines (two HW-DGE rings + SW-DGE) so transfers overlap
            engs = [nc.sync, nc.scalar, nc.gpsimd]
            qi = 0
            nb = len(meta)
            edges = [meta[0][0]] + [m[1] for m in meta]
            for ci in range(nb):
                a, b = edges[ci], edges[ci + 1]
                engs[qi % 3].dma_start(lt[:, a:b, :], lt_d[:, a:b, :])
                qi += 1
                (t0, t1, C1, C2, off0) = meta[ci]
                w = (t1 - t0) * (C1 + 3 + C2)
                engs[qi % 3].dma_start(rhs[:, off0:off0 + w],
                                       rhs_d[:, off0:off0 + w])
                qi += 1

            S1 = singles.tile([P, T], dt_, tag="S1")
            S2 = singles.tile([P, T], dt_, tag="S2")
            nc.vector.memset(S2[:], 0.0)

            for (t0, t1, C1, C2, off0) in meta:
                CW = C1 + 3 + C2
                Bn = t1 - t0
                Wb = Bn * CW
                ps = psum_pool.tile([P, Wb], dt_, tag="ps", name="ps")
                dp = dump_pool.tile([P, Wb], dt_, tag="dp", name="dp")
                for s in range(Bn):
                    for a, b in bank_slices(s * CW, (s + 1) * CW):
                        nc.tensor.matmul(ps[:, a:b], lt[:, t0 + s, :],
                                         rhs[:, off0 + a:off0 + b],
                                         start=True, stop=True)
                nc.scalar.activation(dp[:], ps[:], f.Exp)
                r3 = dp.rearrange("p (b c) -> p b c", c=CW)
                nc.vector.tensor_reduce(S1[:, t0:t1], r3[:, :, 0:C1 + 3],
                                        mybir.AxisListType.X, alu.add)
                if C2 > 0:
                    nc.vector.tensor_reduce(S2[:, t0:t1],
                                            r3[:, :, C1 + 3:CW],
                                            mybir.AxisListType.X, alu.add)

            sd = work.tile([P, T], dt_, tag="sd")
            nc.vector.scalar_tensor_tensor(sd[:], S2[:], -1.0, S1[:],
                                           alu.mult, alu.add)
            # ln + per-partition accumulate in one activation, then a
            # cross-partition reduce; host applies bvec/pad corrections
            lnm = work.tile([P, T], dt_, tag="lnm")
            colsum = singles.tile([P, 1], dt_, tag="colsum")
            nc.scalar.activation(lnm[:], sd[:], f.Ln, accum_out=colsum[:])
            total = singles.tile([P, 1], dt_, tag="total")
            nc.gpsimd.partition_all_reduce(total[:], colsum[:], channels=P,
                                           reduce_op=bass_isa.ReduceOp.add)
            nc.sync.dma_start(out_d.rearrange("(p o) -> p o", p=1),
                              total[0:1, 0:1])

    nc.compile()
    return nc


def kernel(x, y, ku12, ku23, ku13, sigma_b, sigma_n, I1, I2, I3, w):
    pl = _plan(x, y, ku12, ku23, ku13, sigma_b, sigma_n, I1, I2, I3, w)
    in_maps, meta, NT, cores_mask, cores_bvec = _pack(pl, x, y)

    key = (NT, tuple((m[0], m[1], m[2], m[3]) for m in meta))
    if key not in _graph_cache:
        _graph_cache[key] = _build(meta, NT)
    nc = _graph_cache[key]

    res = run_bass_kernel_spmd(nc, in_maps, core_ids=list(range(N_CORES)))
    global _last_results
    _last_results = res

    # device returns sum over all slots of ln(mix'); pads contribute
    # exactly ln(C1'+3-C2') of the last batch each (all-zero planes)
    (t0, t1, C1L, C2L, _o) = meta[-1]
    lnpad = math.log(C1L + 3 - C2L)
    loss = 0.0
    for i in range(N_CORES):
        ts = float(np.asarray(res.results[i]["out"], np.float64)[0])
        npad_i = P * T - float(cores_mask[i].sum())
        bsum_i = float((cores_bvec[i] * cores_mask[i]).sum())
        loss += ts - npad_i * lnpad + bsum_i
    return np.float32(-loss)


# revision 5
# speedup vs baseline: 4.3235x; 1.1161x over previous
"""Trainium2 Bass kernel for the ArcModel3Phase loss (y-sorted redesign).

Math: per point m, logmix = ln(sum_j e^{l_j}) over 6 mixture components
(3 interior Gaussians + 3 MC-integrated interface terms of N=1024 samples
each).  Writing l = A(x,y) + h with A = lny - x^2/2sn^2 - y^2/sn^2 and h
affine in (x, y, lny, 1), every component (and the per-m bias) becomes a
column of ONE bf16 matmul over 17 lhsT rows:

  R[p, c] = sum_k lhsT[k, p] rhs[k, c]   -> exp -> segmented row sums.

Device work per point is ~100 columns instead of 3072 thanks to:
  1. Global y-sort (host permutes; the loss is a sum over m, so no
     unpermute).  Each 1024-point block has a narrow y-range, so most MC
     samples are irrelevant to it: a sample contributes only within
     |y - G(tx)| ~ 0.2.  Host prunes per block against a logmix lower
     bound on an x-grid (cutoff e^-PRUNE).
  2. Adaptive sample merging (2nd-order cumulant, exact variance carried
     as 5 extra matmul rows) with a per-block relevance window, plus an
     overshoot guard that keeps each merged column within OCAP of the
     exact logsumexp at window probes (prevents f32 exp overflow and
     bounds the merge error).
  3. The e^{R2} subtraction pass (Bessel 1-e^{-w} expansion) is skipped
     for samples with w = 4yG/sn^2 >= WSKIP for the whole block - almost
     all of them once y is sorted.
  4. The per-m exp bias nu = b - A (b = max of per-component upper
     bounds, a tight cover of max_j l_j) is pure host math, folded into
     the matmul as two hi/lo bf16 rows.  No on-device max pass at all.
  5. Interior components are affine in (x, lny): 3 more columns, two
     lny rows.  The final ln + masked sum runs on host from the DMA'd
     [128, T] mix tile (f64, more accurate than device f32 accum).

One EXP instruction covers a whole batch of tiles (PSUM budget 2048
f32), then two segmented DVE reduces produce S1 (R1+interior) and S2
per tile; mix = S1 - S2.
"""
import math

import numpy as np
import ml_dtypes
from scipy.special import erf, erfinv

import concourse.bass as bass
import concourse.tile as tile
from concourse import bacc, mybir
from concourse.bass_utils import run_bass_kernel_spmd

BF16 = ml_dtypes.bfloat16
WF = 3.0
LOG2PI = math.log(2.0 * math.pi)
M = 100_000
N_MC = 1024
P = 128
N_CORES = 8
BLK = P * N_CORES              # 1024 points per global block
T = (M + BLK - 1) // BLK       # 98 tiles per core
M_PAD = T * BLK
ROWS = 15
DEAD_B = -30000.0

DM = 24.0                      # max in-window |h - mean| within a group
KMAX = 96                      # max group size
PRUNE = 7.0                    # per-block relevance cutoff (e-folds)
WSKIP = 9.0                    # skip R2 columns with w >= this block-wide
OCAP = 3.5                     # max merged-vs-exact LSE overshoot
XWIN = 0.40                    # merge relevance half-window in x
PSUM_BUDGET = 2048             # f32 columns per batch (4 PSUM banks)
BMAX = 16                      # max tiles per batch

_graph_cache = {}
_last_results = None


def _split(a):
    hi = np.asarray(a).astype(BF16)
    lo = (np.asarray(a, np.float64) - hi.astype(np.float64)).astype(BF16)
    return hi, lo


def _host_rows(ku, Ia, Ib, sigma_b, sigma_n, logw):
    """Raw per-sample rows for one interface term (float64, tx-sorted)."""
    ku = np.asarray(ku, np.float64)
    sn2 = sigma_n ** 2
    I_min = Ia + 0.5 * (Ib - Ia) * (1.0 + erf(-WF / np.sqrt(2.0)))
    I_diff = (Ib - Ia) * erf(WF / np.sqrt(2.0))
    tx = np.sort(ku * I_diff + I_min)
    ei = erfinv(2.0 * (tx - Ia) / (Ib - Ia) - 1.0)
    G = (Ib - Ia) / np.sqrt(2.0 * np.pi * sigma_b ** 2) * np.exp(-ei ** 2)
    lptx = -np.log(2.0 * WF * (Ib - Ia)) + 0.5 * LOG2PI + ei ** 2
    B = -0.5 * tx ** 2 / sn2 - np.log(G) - G ** 2 / sn2 + lptx
    C0 = (-np.log(sigma_n) - 0.5 * LOG2PI
          + np.log(2.0) - 2.0 * np.log(sigma_n)
          + 0.5 * np.log(2.0 / np.pi) - np.log(2.0)
          - 0.5 * np.log(2.0) + np.log(sigma_n))
    Bp = B + np.log(I_diff) - np.log(N_MC) + logw + C0
    return tx, tx / sn2, 2.0 * G / sn2, Bp, G


def _raw_l(xg, yv, term, sn2):
    """l_n(xg, yv) for all samples of one term: [X, N]."""
    tx, txp, g1, Bp, G = term
    w = np.minimum(4.0 * yv * G / sn2, 700.0)
    return (Bp[None, :] + xg[:, None] * txp[None, :] + yv * g1[None, :]
            + np.log1p(-np.exp(-w))[None, :]
            + np.log(yv) - 0.5 * (xg[:, None] ** 2) / sn2 - yv * yv / sn2)


def _interior_logp(x, y, I, sn):
    return (math.log(2.0) + 2.0 * np.log(y) - math.lgamma(1.5)
            - 3.0 * math.log(sn) - (y / sn) ** 2
            - math.log(sn) - 0.5 * LOG2PI - 0.5 * ((x - I) / sn) ** 2)


def _plan(x, y, ku12, ku23, ku13, sigma_b, sigma_n, I1, I2, I3, w):
    x = np.asarray(x, np.float64)
    y = np.asarray(y, np.float64)
    sn = float(sigma_n); sb = float(sigma_b)
    I1, I2, I3 = float(I1), float(I2), float(I3)
    w64 = np.asarray(w, np.float64)
    logw = w64 - (np.log(np.sum(np.exp(w64 - w64.max()))) + w64.max())
    sn2 = sn * sn

    terms = [_host_rows(ku, Ia, Ib, sb, sn, float(logw[3 + j]))
             for j, (ku, Ia, Ib) in enumerate(
                 ((ku12, I1, I2), (ku23, I2, I3), (ku13, I1, I3)))]

    # l(x,y) <= lny + c_u: per-sample peak at (tx, G), minus its lny part
    c_u = -1e30
    for tx, txp, g1, Bp, G in terms:
        l_peak = (np.log(G) + 0.5 * tx ** 2 / sn2 + G ** 2 / sn2 + Bp
                  + np.log1p(-np.exp(-np.minimum(4.0 * G * G / sn2, 700.0))))
        c_u = max(c_u, float((l_peak - np.log(G)).max()))

    order = np.argsort(y, kind="stable")
    pad = M_PAD - len(x)
    order_p = np.concatenate([order, np.repeat(order[-1], pad)])
    mask_p = np.concatenate([np.ones(len(x), np.float32),
                             np.zeros(pad, np.float32)])
    ys = y[order_p]

    xmin, xmax = float(x.min()), float(x.max())
    xg = np.linspace(xmin, xmax, 121)

    def logmix_lb(yv):
        mx = np.maximum.reduce([_interior_logp(xg, yv, I, sn) + logw[k]
                                for k, I in enumerate((I1, I2, I3))])
        for term in terms:
            l = _raw_l(xg, yv, term, sn2)
            m2 = l.max(axis=1)
            mx = np.maximum(mx, m2 + np.log(
                np.sum(np.exp(l - m2[:, None]), axis=1)))
        return mx

    tiles = []
    for t in range(T):
        blk = slice(t * BLK, (t + 1) * BLK)
        yb = ys[blk]
        ylo, yhi = float(yb.min()), float(yb.max())
        yprobes = np.linspace(ylo, yhi, 3)
        lmix = np.max([logmix_lb(yv) for yv in yprobes], axis=0)

        tile_terms = []
        for term in terms:
            tx, txp, g1, Bp, G = term
            keep = np.zeros(len(tx), bool)
            for yv in yprobes:
                l = _raw_l(xg, yv, term, sn2)
                keep |= (l - lmix[:, None] >= -PRUNE).any(axis=0)
            yc = np.clip(G, ylo, yhi)
            wv = np.minimum(4.0 * yc * G / sn2, 700.0)
            l = (Bp[None, :] + xg[:, None] * txp[None, :]
                 + (yc * g1)[None, :] + np.log1p(-np.exp(-wv))[None, :]
                 + np.log(yc)[None, :] - 0.5 * (xg[:, None] ** 2) / sn2
                 - (yc * yc)[None, :] / sn2)
            keep |= (l - lmix[:, None] >= -PRUNE).any(axis=0)

            idx = np.where(keep)[0]
            if len(idx) == 0:
                tile_terms.append(None)
                continue
            txk, txpk, g1k, Bpk, Gk = (tx[idx], txp[idx], g1[idx],
                                       Bp[idx], G[idx])
            groups = []
            i = 0
            n = len(idx)
            while i < n:
                k = min(KMAX, n - i)
                while k > 1:
                    tt, gg, bb = txpk[i:i + k], g1k[i:i + k], Bpk[i:i + k]
                    txm = txk[i:i + k].mean()
                    lo = max(xmin, txm - XWIN); hi = min(xmax, txm + XWIN)
                    dt = tt - tt.mean(); db = bb - bb.mean()
                    dg = np.abs(gg - gg.mean())
                    d = (np.maximum(np.abs(dt * lo + db),
                                    np.abs(dt * hi + db)) + dg * yhi)
                    if d.max() <= DM:
                        xp = np.array([lo, 0.5 * (lo + hi), hi])
                        yp = np.array([ylo, yhi])
                        ok = True
                        for sgn in (1.0, -1.0):
                            h = (xp[:, None, None] * tt[None, None, :]
                                 + sgn * yp[None, :, None] * gg[None, None, :]
                                 + bb[None, None, :])
                            mu = h.mean(axis=2)
                            var = h.var(axis=2)
                            mx = h.max(axis=2)
                            lse = mx + np.log(
                                np.exp(h - mx[:, :, None]).sum(axis=2))
                            if ((mu + var / 2.0 + math.log(k) - lse).max()
                                    > OCAP):
                                ok = False
                                break
                        if ok:
                            break
                    k = k - 1 if k <= 4 else int(k * 0.7)
                k = max(k, 1)
                groups.append((i, k))
                i += k
            mg = []
            for i0, k in groups:
                tt, gg, bb = txpk[i0:i0 + k], g1k[i0:i0 + k], Bpk[i0:i0 + k]
                mg.append((tt.mean(), gg.mean(),
                           bb.mean() + math.log(k) + bb.var() / 2.0,
                           tt.var() / 2.0, gg.var() / 2.0,
                           np.mean((tt - tt.mean()) * (gg - gg.mean())),
                           np.mean((tt - tt.mean()) * (bb - bb.mean())),
                           np.mean((gg - gg.mean()) * (bb - bb.mean())),
                           Gk[i0:i0 + k].min()))
            cols = [np.array(v) for v in zip(*mg)]
            keep2 = cols[8] * 4.0 * max(ylo, 1e-9) / sn2 < WSKIP
            tile_terms.append((cols, keep2))

        c1 = sum(len(tt[0][0]) for tt in tile_terms if tt)
        c2 = sum(int(tt[1].sum()) for tt in tile_terms if tt)
        tiles.append({"ylo": ylo, "yhi": yhi, "terms": tile_terms,
                      "c1": c1, "c2": c2})

    batches = []
    bstart = 0
    while bstart < T:
        bend = bstart + 1
        best = bstart + 1
        while bend <= T:
            c1m = max(tl["c1"] for tl in tiles[bstart:bend])
            c2m = max(tl["c2"] for tl in tiles[bstart:bend])
            Bn = bend - bstart
            if Bn * (c1m + 3 + c2m) > PSUM_BUDGET or Bn > BMAX:
                break
            best = bend
            bend += 1
        bend = best
        c1m = max(tl["c1"] for tl in tiles[bstart:bend])
        c2m = max(tl["c2"] for tl in tiles[bstart:bend])
        batches.append((bstart, bend, c1m, c2m))
        bstart = bend

    return {"order_p": order_p, "mask_p": mask_p, "tiles": tiles,
            "batches": batches, "logw": logw, "sn": sn, "c_u": c_u,
            "I": (I1, I2, I3)}


def _term_block(cols, sel, sgn):
    """rhs columns [ROWS, n] for one term's merged samples.  The ctb/cgb
    covariance corrections are folded into the t/g rows before the hi/lo
    split (they pair with the same x/y lhsT planes)."""
    tm, gm, Bm, vt, vg, ctg, ctb, cgb, _g = cols
    th, tl = _split(tm[sel] + ctb[sel])
    gh, gl = _split(sgn * (gm[sel] + cgb[sel]))
    bh, bl = _split(Bm[sel])
    n = len(th)
    one = np.ones(n, BF16)
    z = np.zeros(n, BF16)
    return np.stack([
        th, tl, th, gh, gl, gh, bh, bl,
        vt[sel].astype(BF16), vg[sel].astype(BF16),
        (sgn * ctg[sel]).astype(BF16),
        one, one, z, z,
    ]).astype(BF16)


def _pack(pl, x, y):
    """Build rhs [ROWS, NT] (shared), per-core lt [ROWS,T,P], mask, bvec."""
    sn = pl["sn"]; sn2 = sn * sn
    I1, I2, I3 = pl["I"]
    logw = pl["logw"]
    xs = np.asarray(x, np.float64)[pl["order_p"]]
    ysrt = np.asarray(y, np.float64)[pl["order_p"]]

    lny = np.log(ysrt)
    A = lny - 0.5 * (xs / sn) ** 2 - (ysrt / sn) ** 2
    Ps = [_interior_logp(xs, ysrt, I, sn) + logw[k]
          for k, I in enumerate((I1, I2, I3))]
    b_m = np.maximum.reduce(Ps + [lny + pl["c_u"]])
    nu64 = b_m - A
    nmh, nml = _split(-nu64)
    nu_use = -(nmh.astype(np.float64) + nml.astype(np.float64))
    bvec = (A + nu_use)
    lh, ll = _split(lny)

    ck = (math.log(2.0) - math.lgamma(1.5) - 4.0 * math.log(sn)
          - 0.5 * LOG2PI)
    intcols = np.zeros((ROWS, 3), BF16)
    for k, I in enumerate((I1, I2, I3)):
        tih, til = _split(np.array([I / sn2]))
        csth, cstl = _split(np.array([ck + logw[k] - 0.5 * I * I / sn2]))
        col = np.zeros(ROWS, BF16)
        col[0], col[1], col[2] = tih[0], til[0], tih[0]
        col[6], col[7] = csth[0], cstl[0]
        col[11] = col[12] = col[13] = col[14] = BF16(1.0)
        intcols[:, k] = col
    deadcol = np.zeros(ROWS, BF16)
    deadcol[6] = BF16(DEAD_B)

    # rhs: batch-major, per tile [C1' R1-cols | 3 interior | C2' R2-cols]
    rhs_parts = []
    meta = []
    off = 0
    for (t0, t1, C1, C2) in pl["batches"]:
        CW = C1 + 3 + C2
        for t in range(t0, t1):
            tt = pl["tiles"][t]
            blocks = []
            n1 = 0
            for j in range(3):
                ter = tt["terms"][j]
                if ter is None:
                    continue
                cols, keep2 = ter
                blk = _term_block(cols, slice(None), 1.0)
                blocks.append(blk)
                n1 += blk.shape[1]
            if n1 < C1:
                blocks.append(np.repeat(deadcol[:, None], C1 - n1, axis=1))
            blocks.append(intcols)
            n2 = 0
            for j in range(3):
                ter = tt["terms"][j]
                if ter is None:
                    continue
                cols, keep2 = ter
                if keep2.any():
                    blk = _term_block(cols, np.where(keep2)[0], -1.0)
                    blocks.append(blk)
                    n2 += blk.shape[1]
            if n2 < C2:
                blocks.append(np.repeat(deadcol[:, None], C2 - n2, axis=1))
            rhs_parts.append(np.concatenate(blocks, axis=1))
        meta.append((t0, t1, C1, C2, off))
        off += (t1 - t0) * CW
    rhs = np.concatenate(rhs_parts, axis=1).astype(BF16)
    NT = rhs.shape[1]
    assert NT == off

    xh, xl = _split(xs); yh, yl = _split(ysrt)
    planes = [
        xh, xh, xl, yh, yh, yl,
        np.ones(M_PAD, BF16), np.ones(M_PAD, BF16),
        (xs * xs).astype(BF16), (ysrt * ysrt).astype(BF16),
        (xs * ysrt).astype(BF16), nmh, nml, lh, ll,
    ]
    # pad slots: all-zero planes -> R = 0 for every column -> the slot
    # contributes exactly ln(C1'+3-C2') to the device sum (host-corrected)
    padm = pl["mask_p"] == 0.0
    planes = [np.where(padm, np.zeros(1, BF16), p).astype(BF16)
              for p in planes]

    # safety: packed R - nu must stay well under f32 exp overflow
    pf = np.stack([p.astype(np.float32) for p in planes]).astype(np.float64)
    rf = rhs.astype(np.float64)
    vmax = -1e30
    for (t0, t1, C1, C2, off0) in meta:
        CW = C1 + 3 + C2
        for s, t in enumerate(range(t0, t1)):
            blk = slice(t * BLK, (t + 1) * BLK)
            Rt = pf[:, blk].T @ rf[:, off0 + s * CW: off0 + (s + 1) * CW]
            vmax = max(vmax, float(Rt.max()))
    assert vmax < 60.0, f"exp overflow risk: max(R-nu)={vmax:.1f}"

    in_maps = []
    cores_bvec = []
    cores_mask = []
    for i in range(N_CORES):
        lt = np.empty((ROWS, T, P), BF16)
        for r in range(ROWS):
            lt[r] = planes[r].reshape(T, N_CORES, P)[:, i, :]
        mask_i = np.ascontiguousarray(
            pl["mask_p"].reshape(T, N_CORES, P)[:, i, :].T).astype(np.float32)
        bvec_i = bvec.reshape(T, N_CORES, P)[:, i, :].T
        in_maps.append({"lt": lt, "rhs": rhs})
        cores_mask.append(mask_i)
        cores_bvec.append(bvec_i)
    return in_maps, meta, NT, cores_mask, cores_bvec


def _build(meta, NT):
    nc = bacc.Bacc("TRN2", target_bir_lowering=False, debug=False,
                   num_devices=N_CORES)
    dt_ = mybir.dt.float32
    bf = mybir.dt.bfloat16
    f = mybir.ActivationFunctionType
    alu = mybir.AluOpType

    lt_d = nc.dram_tensor("lt", [ROWS, T, P], bf, kind="ExternalInput").ap()
    rhs_d = nc.dram_tensor("rhs", [ROWS, NT], bf, kind="ExternalInput").ap()
    out_d = nc.dram_tensor("out", [P, T], dt_, kind="ExternalOutput").ap()

    def bank_slices(a, b):
        out = []
        while a < b:
            c = min(b, (a // 512 + 1) * 512)
            out.append((a, c))
            a = c
        return out

    with tile.TileContext(nc) as tc:
        with (
            tc.tile_pool(name="singles", bufs=1) as singles,
            tc.tile_pool(name="work", bufs=2) as work,
            tc.tile_pool(name="psum", bufs=2, space="PSUM") as psum_pool,
            tc.tile_pool(name="dump", bufs=2) as dump_pool,
        ):
            lt = singles.tile([ROWS, T, P], bf, tag="lt")
            rhs = singles.tile([ROWS, NT], bf, tag="rhs")
            # chunked loads, round-robin across the three DMA-capable
            # engines (two HW-DGE rings + SW-DGE) so transfers overlap;
            # batch 0's chunks go first on the HW rings so compute starts
            engs = [nc.sync, nc.scalar, nc.gpsimd]
            qi = 2
            nb = len(meta)
            edges = [meta[0][0]] + [m[1] for m in meta]
            for ci in range(nb):
                a, b = edges[ci], edges[ci + 1]
                (t0, t1, C1, C2, off0) = meta[ci]
                w = (t1 - t0) * (C1 + 3 + C2)
                if ci == 0:
                    nc.sync.dma_start(lt[:, a:b, :], lt_d[:, a:b, :])
                    nc.scalar.dma_start(rhs[:, off0:off0 + w],
                                        rhs_d[:, off0:off0 + w])
                    continue
                engs[qi % 3].dma_start(lt[:, a:b, :], lt_d[:, a:b, :])
                qi += 1
                engs[qi % 3].dma_start(rhs[:, off0:off0 + w],
                                       rhs_d[:, off0:off0 + w])
                qi += 1

            S1 = singles.tile([P, T], dt_, tag="S1")
            S2 = singles.tile([P, T], dt_, tag="S2")
            nc.vector.memset(S2[:], 0.0)

            for (t0, t1, C1, C2, off0) in meta:
                CW = C1 + 3 + C2
                Bn = t1 - t0
                Wb = Bn * CW
                ps = psum_pool.tile([P, Wb], dt_, tag="ps", name="ps")
                dp = dump_pool.tile([P, Wb], dt_, tag="dp", name="dp")
                for s in range(Bn):
                    for a, b in bank_slices(s * CW, (s + 1) * CW):
                        nc.tensor.matmul(ps[:, a:b], lt[:, t0 + s, :],
                                         rhs[:, off0 + a:off0 + b],
                                         start=True, stop=True)
                nc.scalar.activation(dp[:], ps[:], f.Exp)
                r3 = dp.rearrange("p (b c) -> p b c", c=CW)
                nc.vector.tensor_reduce(S1[:, t0:t1], r3[:, :, 0:C1 + 3],
                                        mybir.AxisListType.X, alu.add)
                if C2 > 0:
                    nc.vector.tensor_reduce(S2[:, t0:t1],
                                            r3[:, :, C1 + 3:CW],
                                            mybir.AxisListType.X, alu.add)

            sd = work.tile([P, T], dt_, tag="sd")
            nc.vector.scalar_tensor_tensor(sd[:], S2[:], -1.0, S1[:],
                                           alu.mult, alu.add)
            nc.sync.dma_start(out_d, sd[:])

    nc.compile()
    return nc


def kernel(x, y, ku12, ku23, ku13, sigma_b, sigma_n, I1, I2, I3, w):
    pl = _plan(x, y, ku12, ku23, ku13, sigma_b, sigma_n, I1, I2, I3, w)
    in_maps, meta, NT, cores_mask, cores_bvec = _pack(pl, x, y)

    key = (NT, tuple((m[0], m[1], m[2], m[3]) for m in meta))
    if key not in _graph_cache:
        _graph_cache[key] = _build(meta, NT)
    nc = _graph_cache[key]

    res = run_bass_kernel_spmd(nc, in_maps, core_ids=list(range(N_CORES)))
    global _last_results
    _last_results = res

    loss = 0.0
    for i in range(N_CORES):
        sd = np.asarray(res.results[i]["out"], np.float64)
        lm = np.log(np.maximum(sd, 1e-300)) + cores_bvec[i]
        loss += float((lm * cores_mask[i]).sum())
    return np.float32(-loss)
